# revision 30
# baseline (speedup 1.0000x reference)
"""Masked causal self-attention (single head) on 8 Trainium2 NeuronCores.

Problem: x[4,4096,1024], mask[4,4096] (key padding), Wq/Wk/Wv[128,1024],
bq/bk/bv[128] -> out[4,4096,128]:
    q = x@Wq.T+bq; k = x@Wk.T+bk; v = x@Wv.T+bv
    out = softmax(causal_mask(q@k.T/sqrt(128)) + key_padding) @ v

Sharding (SPMD, one program on 8 cores): core c = (batch b=c//2, parity
p=c%2). Each core computes K/V for its full batch and handles the
interleaved query 128-row tiles {2*t+p : t in 0..15} — interleaving
balances the causal (triangular) work between the pair.

To keep the program core-uniform, the host PERMUTES x's sequence tiles per
core so the core's own query tiles always sit at EVEN 128-column positions
(p=0: natural order; p=1: pairwise swap). All causal structure is then
position-uniform; the one residual parity difference (whether the odd
neighbor tile is a past or future key) is data (maskB2 below).

Device algorithm per core (everything on the PE runs in bf16, 1 cycle/row
at any moving size; PSUM accumulates fp32):
  - x streams in as one batched DMA per 512-column chunk ([128, 8, 512]
    d-major tile). K^T/V^T/Q^T project with the d-contraction on
    partitions, accumulating over 8 d-chunks in PSUM; Q uses the same
    resident x tiles (own tiles = even 128-blocks, one strided-AP matmul)
    so x is read exactly once. Biases fold in during PSUM->SBUF eviction;
    the 1/sqrt(128) score scale is folded into Wq/bq on the host.
  - V^T transposes back to [key, head] via 4 PE transposes sharing one
    PSUM accumulation region, then one strided DVE copy into the Vm
    buffer whose 129th column is a constant 1 (memset once).
  - Scores S^T[k,q] = KT-tile.T @ QT-chunk. The causal mask is applied
    INSIDE the score accumulation group on the PE: for the two edge key
    tiles of each query block, a matmul with stationary -1e30*triangle
    (resp. the parity mask) and identity moving adds -1e30 above the
    diagonal. exp() then runs on the scalar engine straight out of PSUM
    with the key-padding bias per partition; its only dependency is the
    PE, so no extra semaphore ops are legalized in. Softmax
    max-subtraction is skipped: scores are ~N(0,1) by construction.
  - attn@V runs with the exp tile as the STATIONARY operand and
    [V | ones] (129 cols) as the moving operand: out[q,128:129] then IS
    the softmax denominator, accumulated for free, and the output comes
    out in [q, head] orientation. Two accumulation groups share each PSUM
    bank (single start/stop per bank; PSUM pending-zero semantics make
    the second group's first touch an overwrite).
  - Normalization is reciprocal + per-partition scalar multiply.
  - Projections for chunk j+1 are emission-interleaved into attention
    chunk j so the PE has filler while the scalar engine exps.
"""

import sys

sys.path.insert(0, "/opt/trn_rl_repo")

import numpy as np
import ml_dtypes

import concourse.bass as bass
import concourse.bacc as bacc
import concourse.tile as tile
from concourse import mybir
from concourse import bass_utils

F32 = mybir.dt.float32
BF16 = mybir.dt.bfloat16
B, S, D, H = 4, 4096, 1024, 128
NQ = S // 2          # queries owned per core (2048)
DC = D // 128        # 8 d-chunks
NSC = S // 512       # 8 seq chunks of 512
NKT = S // 128       # 32 key tile positions
NJ = 4               # attention chunks of 512 owned queries
VW = 132             # Vm row pitch (129 used)


def _build_program():
    nc = bacc.Bacc("TRN2", target_bir_lowering=False)

    xT_d = nc.dram_tensor("xT", [D, S], BF16, kind="ExternalInput")
    wqT_d = nc.dram_tensor("wqT", [128, DC * H], BF16, kind="ExternalInput")
    wkT_d = nc.dram_tensor("wkT", [128, DC * H], BF16, kind="ExternalInput")
    wvT_d = nc.dram_tensor("wvT", [128, DC * H], BF16, kind="ExternalInput")
    bq_d = nc.dram_tensor("bq", [H, 1], F32, kind="ExternalInput")
    bk_d = nc.dram_tensor("bk", [H, 1], F32, kind="ExternalInput")
    bv_d = nc.dram_tensor("bv", [H, 1], F32, kind="ExternalInput")
    ident_d = nc.dram_tensor("ident", [128, 128], BF16, kind="ExternalInput")
    triM_d = nc.dram_tensor("triM", [128, 128], BF16, kind="ExternalInput")
    mb2_d = nc.dram_tensor("maskB2", [128, 128], BF16, kind="ExternalInput")
    mb_d = nc.dram_tensor("mb", [128, NKT], F32, kind="ExternalInput")
    o_d = nc.dram_tensor("o", [NQ, H], F32, kind="ExternalOutput")

    with tile.TileContext(nc) as tc:
        with (
            tc.tile_pool(name="consts", bufs=1) as consts,
            tc.tile_pool(name="big", bufs=1) as big,
            tc.tile_pool(name="xp", bufs=7) as xp,
            tc.tile_pool(name="vstage", bufs=2) as vstage,
            tc.tile_pool(name="ptp", bufs=8) as ptp,
            tc.tile_pool(name="osb", bufs=4) as osb,
            tc.tile_pool(name="rp", bufs=4) as rp,
            tc.tile_pool(name="kvps", bufs=1, space="PSUM") as kvps,
            tc.tile_pool(name="scr", bufs=1, space="PSUM") as scr,
            tc.tile_pool(name="qps", bufs=1, space="PSUM") as qps,
            tc.tile_pool(name="sp", bufs=2, space="PSUM") as sp,
            tc.tile_pool(name="op", bufs=2, space="PSUM") as op,
        ):
            # ---- weights on the gpsimd queue; sync carries only x tiles so
            # the first x chunk starts transferring as early as possible ----
            w_sb = {}
            for name, dram in (("k", wkT_d), ("v", wvT_d), ("q", wqT_d)):
                t = consts.tile([128, DC, H], BF16, tag=f"w_{name}")
                nc.gpsimd.dma_start(out=t, in_=dram[:, :].rearrange("p (c h) -> p c h", c=DC))
                w_sb[name] = t
            # ---- small consts on the gpsimd queue (off critical path) ----
            ident = consts.tile([128, 128], BF16)
            nc.gpsimd.dma_start(out=ident, in_=ident_d[:, :])
            triM = consts.tile([128, 128], BF16)
            nc.gpsimd.dma_start(out=triM, in_=triM_d[:, :])
            maskB2 = consts.tile([128, 128], BF16)
            nc.gpsimd.dma_start(out=maskB2, in_=mb2_d[:, :])
            mb = consts.tile([128, NKT], F32)
            nc.gpsimd.dma_start(out=mb, in_=mb_d[:, :])
            b_sb = {}
            for name, dram in (("q", bq_d), ("k", bk_d), ("v", bv_d)):
                t = consts.tile([H, 1], F32, tag=f"b_{name}")
                nc.gpsimd.dma_start(out=t, in_=dram[:, :])
                b_sb[name] = t

            KT = big.tile([128, S], BF16, tag="KT")      # K^T [h, kpos]
            QT = big.tile([128, NQ], BF16, tag="QT")     # Q^T [h, own q]
            Vm = big.tile([128, NKT, VW], BF16, tag="Vm")  # [k, h | ones]

            # PE p-state warmup: the tensor engine only reaches full clock
            # after ~3us of continuous execution. Run garbage matmuls while
            # the first x chunk is still in flight so the real work starts
            # at speed. (Reads an unwritten tile; the result is discarded.)
            wsrc = big.tile([128, 512], BF16, tag="wsrc")
            nc.vector.memset(wsrc, 0.0)
            nc.vector.memset(Vm[:, :, 128:129], 1.0)
            for _ in range(18):
                wdst = sp.tile([128, 512], F32, tag="sp", name="wdst")
                nc.tensor.matmul(wdst, wsrc[:, 0:128], wsrc,
                                 start=True, stop=True)

            # ---- projection emission units ----
            # one unit = one (s-chunk, d-chunk) step or an eviction step;
            # attention chunks interleave these to keep the PE fed.
            cur = {}

            def proj_dma(sc, split=False):
                if split:
                    # dc-pair DMAs so the first matmul can start after ~1/4
                    # of the chunk has landed
                    tiles = []
                    for h in range(4):
                        xb = xp.tile([128, 2, 512], BF16, tag=f"xs{h}",
                                     name=f"xs{h}", bufs=1)
                        nc.sync.dma_start(
                            out=xb,
                            in_=xT_d[h * 256:(h + 1) * 256,
                                     sc * 512:(sc + 1) * 512].rearrange(
                                "(c p) s -> p c s", p=128),
                        )
                        tiles.append(xb)
                    cur[sc] = lambda dc: tiles[dc // 2][:, dc % 2, :]
                else:
                    xb = xp.tile([128, DC, 512], BF16, tag="xt")
                    eng = nc.sync if sc % 2 == 0 else nc.scalar
                    eng.dma_start(
                        out=xb,
                        in_=xT_d[:, sc * 512:(sc + 1) * 512].rearrange(
                            "(c p) s -> p c s", p=128),
                    )
                    cur[sc] = lambda dc, xb=xb: xb[:, dc, :]

            def proj_step(sc, dc):
                if dc == 0:
                    cur["kv"] = kvps.tile([128, 1024], F32, name="kvp")
                    cur["q"] = qps.tile([128, 256], F32, name="qp")
                kvp, qp = cur["kv"], cur["q"]
                xt = cur[sc](dc)
                st, sp_ = (dc == 0), (dc == DC - 1)
                nc.tensor.matmul(kvp[:, 0:512], w_sb["k"][:, dc, :], xt,
                                 start=st, stop=sp_)
                nc.tensor.matmul(kvp[:, 512:1024], w_sb["v"][:, dc, :], xt,
                                 start=st, stop=sp_)
                # own query tiles sit at even 128-positions: cols 0:128, 256:384
                xq = bass.AP(tensor=xt.tensor, offset=xt.offset,
                             ap=[list(xt.ap[0]), [256, 2], [1, 128]])
                nc.tensor.matmul(qp, w_sb["q"][:, dc, :], xq,
                                 start=st, stop=sp_)

            def proj_evict(sc):
                kvp, qp = cur["kv"], cur["q"]
                nc.vector.tensor_scalar_add(
                    KT[:, sc * 512:(sc + 1) * 512], kvp[:, 0:512], b_sb["k"])
                vst = vstage.tile([128, 512], BF16, tag="vst")
                nc.vector.tensor_scalar_add(vst, kvp[:, 512:1024], b_sb["v"])
                nc.vector.tensor_scalar_add(
                    QT[:, sc * 256:(sc + 1) * 256], qp, b_sb["q"])
                cur["vst"] = vst

            def proj_vt(sc, i):
                # 4 transposes share one PSUM region (single start/stop group)
                if i == 0:
                    cur["tq"] = scr.tile([128, 512], BF16, name="tq")
                nc.tensor.matmul(
                    cur["tq"][:, i * 128:(i + 1) * 128],
                    cur["vst"][:, i * 128:(i + 1) * 128], ident,
                    is_transpose=True, start=(i == 0), stop=(i == 3))
                if i == 3:
                    nc.vector.tensor_copy(
                        Vm[:, 4 * sc:4 * sc + 4, 0:128],
                        cur["tq"].rearrange("p (a b) -> p a b", a=4))

            def proj_units(sc):
                for dc in range(DC):
                    yield lambda dc=dc: proj_step(sc, dc)
                yield lambda: proj_evict(sc)
                for i in range(4):
                    yield lambda i=i: proj_vt(sc, i)

            # ---- attention chunk j over owned query cols [512j, 512j+512) ----
            # q-block qi (0..3) is own tile t=4j+qi at key-position 8j+2qi;
            # PV for (qi, kt) needed for kt <= 8j+2qi+1. The causal edge is
            # applied pre-exp on the PE: kt==8j+2qi adds -1e30 above the
            # diagonal (triM), kt==8j+2qi+1 adds the parity mask (maskB2:
            # -1e30 everywhere for p=0 where the neighbor is a future key,
            # 0 for p=1 where it is a past key).
            def attention(j, filler):
                n_kt = 8 * j + 8
                opsA = op.tile([128, 258], F32, tag="o2")   # qi 0,1
                opsB = op.tile([128, 258], F32, tag="o2")   # qi 2,3
                pts = [None] * n_kt

                def score_exp(kt):
                    hi_only = kt > 8 * j + 3
                    w = 256 if hi_only else 512
                    qoff = j * 512 + (256 if hi_only else 0)
                    edge = kt >= 8 * j
                    spsum = sp.tile([128, 512], F32, tag="sp")
                    nc.tensor.matmul(
                        spsum[:, 0:w], KT[:, kt * 128:(kt + 1) * 128],
                        QT[:, qoff:qoff + w], start=True, stop=not edge)
                    if edge:
                        qi_e, c = (kt - 8 * j) // 2, (kt - 8 * j) % 2
                        lo = qi_e * 128 - (256 if hi_only else 0)
                        nc.tensor.matmul(
                            spsum[:, lo:lo + 128],
                            triM if c == 0 else maskB2, ident,
                            start=False, stop=True)
                    pt = ptp.tile([128, 512], BF16, tag="pt")
                    nc.scalar.activation(
                        pt[:, 0:w], spsum[:, 0:w],
                        mybir.ActivationFunctionType.Exp,
                        bias=mb[:, kt:kt + 1], scale=1.0)
                    pts[kt] = pt

                def pv(kt):
                    hi_only = kt > 8 * j + 3
                    qi_min = max(0, -(-(kt - 8 * j - 1) // 2))
                    for qi in range(qi_min, 4):
                        lo = qi * 128 - (256 if hi_only else 0)
                        ops = opsA if qi < 2 else opsB
                        col = (qi % 2) * 129
                        nc.tensor.matmul(
                            ops[:, col:col + 129], pts[kt][:, lo:lo + 128],
                            Vm[:, kt, 0:129],
                            start=(kt == 0 and qi % 2 == 0),
                            stop=(qi == 1 and kt == 8 * j + 3)
                            or (qi == 3 and kt == n_kt - 1),
                        )

                def epilogue(qi):
                    ops = opsA if qi < 2 else opsB
                    col = (qi % 2) * 129
                    r = rp.tile([128, 1], F32, tag="r")
                    nc.vector.reciprocal(r, ops[:, col + 128:col + 129])
                    o_sb = osb.tile([128, 128], F32, tag="o")
                    nc.vector.tensor_scalar_mul(o_sb, ops[:, col:col + 128], r)
                    row = (4 * j + qi) * 128
                    nc.sync.dma_start(out=o_d[row:row + 128, :], in_=o_sb)

                # lag-2 software pipeline: pv(kt) runs two score steps after
                # score_exp(kt), so the PE never waits on the scalar engine's
                # exp latency in steady state. Filler absorbs the one
                # score(2)-waits-exp(0) PSUM-reuse bubble at chunk start.
                score_exp(0)
                score_exp(1)
                for f in filler.pop_units(2):
                    f()
                for kt in range(2, n_kt):
                    score_exp(kt)
                    for f in filler.pop_units(4 if j == 0 else 2):
                        f()
                    pv(kt - 2)
                    if kt - 2 == 8 * j + 3:
                        epilogue(0)
                        epilogue(1)
                    elif kt - 2 == 8 * j + 5:
                        epilogue(2)
                pv(n_kt - 2)
                pv(n_kt - 1)
                epilogue(3)

            class Filler:
                def __init__(self):
                    self.units = []

                def add(self, sc):
                    self.units.extend(proj_units(sc))

                def pop_units(self, k):
                    for _ in range(k):
                        if self.units:
                            yield self.units.pop(0)

                def drain(self):
                    yield from self.pop_units(len(self.units))

            filler = Filler()
            proj_dma(0, split=True)
            for sc in range(1, NSC):
                proj_dma(sc)
            filler.add(0)
            filler.add(1)
            for f in filler.drain():
                f()
            for j in range(NJ):
                if 2 * j + 2 < NSC:
                    filler.add(2 * j + 2)
                    filler.add(2 * j + 3)
                attention(j, filler)
                for f in filler.drain():
                    f()
    nc.compile()
    return nc


_NC_CACHE = {}


def _get_program():
    if "nc" not in _NC_CACHE:
        _NC_CACHE["nc"] = _build_program()
    return _NC_CACHE["nc"]


def _make_in_maps(x, mask, Wq, bq, Wk, bk, Wv, bv):
    x = np.asarray(x, np.float32)
    mask = np.asarray(mask)
    scale = 1.0 / np.sqrt(np.float32(H))
    bf16 = ml_dtypes.bfloat16
    NEG = np.float32(-1.0e30)

    def pack_w(w):
        # [H,D] -> w.T [D,H] -> partition-major [128, DC*H] for a single
        # contiguous-burst DMA into the SBUF weight tile
        wT = np.asarray(w, np.float32).T.reshape(DC, 128, H)
        return np.ascontiguousarray(
            wT.transpose(1, 0, 2).reshape(128, DC * H).astype(bf16))

    wqT = pack_w(np.asarray(Wq, np.float32) * scale)
    wkT = pack_w(Wk)
    wvT = pack_w(Wv)
    bq_c = (np.asarray(bq, np.float32) * scale).reshape(H, 1).copy()
    bk_c = np.asarray(bk, np.float32).reshape(H, 1).copy()
    bv_c = np.asarray(bv, np.float32).reshape(H, 1).copy()
    ident = np.eye(128, dtype=bf16)
    # score += triM.T[k, q']: -1e30 where q' < k (strict upper as [q', k])
    triM = (NEG * np.triu(np.ones((128, 128), np.float32), 1)).astype(bf16)

    in_maps = []
    for c in range(8):
        b, p = c // 2, c % 2
        # permuted tile order: even positions = own tiles (parity p)
        perm = np.arange(NKT).reshape(-1, 2)
        if p == 1:
            perm = perm[:, ::-1]
        perm = perm.reshape(-1)                                # pos -> global tile
        xT = x[b].T.reshape(D, NKT, 128)[:, perm, :].reshape(D, S)
        maskB2 = (np.full((128, 128), NEG) if p == 0
                  else np.zeros((128, 128), np.float32))
        mb = np.where(mask[b] != 0, np.float32(0.0), NEG)
        mb = np.ascontiguousarray(mb.reshape(NKT, 128)[perm, :].T)
        in_maps.append({
            "xT": np.ascontiguousarray(xT.astype(bf16)),
            "wqT": wqT, "wkT": wkT, "wvT": wvT,
            "bq": bq_c, "bk": bk_c, "bv": bv_c,
            "ident": ident, "triM": triM, "maskB2": maskB2.astype(bf16),
            "mb": mb,
        })
    return in_maps


def _install_ntff_hook():
    # the trimmed antenv package lacks axon_hooks; recreate it and wire the
    # ctypes NTFF profiling hook from trn_agent_boot so trace=True works
    import types
    if "antenv.axon_hooks" in sys.modules:
        return
    import antenv
    mod = types.ModuleType("antenv.axon_hooks")
    _hook = [None]
    mod.set_axon_ntff_profile_hook = lambda h: _hook.__setitem__(0, h)
    mod.get_axon_ntff_profile_hook = lambda: _hook[0]
    sys.modules["antenv.axon_hooks"] = mod
    antenv.axon_hooks = mod
    from trn_agent_boot.trn_boot import _ntff_profile_via_ctypes
    mod.set_axon_ntff_profile_hook(
        _ntff_profile_via_ctypes("/opt/axon/libaxon_pjrt.so"))


def run(inputs, trace=False, tmpdir=None):
    if trace:
        try:
            _install_ntff_hook()
        except Exception as e:
            print("ntff hook install failed:", e)
    nc = _get_program()
    in_maps = _make_in_maps(**inputs)
    res = bass_utils.run_bass_kernel_spmd(
        nc, in_maps, core_ids=list(range(8)), trace=trace, tmpdir=tmpdir)
    out = np.empty((B, S, H), np.float32)
    for c in range(8):
        b, p = c // 2, c % 2
        o = res.results[c]["o"]                                # [NQ, H]
        for t in range(16):
            g = 2 * t + p
            out[b, g * 128:(g + 1) * 128, :] = o[t * 128:(t + 1) * 128, :]
    return out, res


def kernel(**inputs) -> np.ndarray:
    out, _ = run(inputs, trace=False)
    return out


# revision 31
# speedup vs baseline: 1.0211x; 1.0211x over previous
"""Masked causal self-attention (single head) on 8 Trainium2 NeuronCores.

Problem: x[4,4096,1024], mask[4,4096] (key padding), Wq/Wk/Wv[128,1024],
bq/bk/bv[128] -> out[4,4096,128]:
    q = x@Wq.T+bq; k = x@Wk.T+bk; v = x@Wv.T+bv
    out = softmax(causal_mask(q@k.T/sqrt(128)) + key_padding) @ v

Sharding (SPMD, one program on 8 cores): core c = (batch b=c//2, parity
p=c%2). Each core computes K/V for its full batch and handles the
interleaved query 128-row tiles {2*t+p : t in 0..15} — interleaving
balances the causal (triangular) work between the pair.

To keep the program core-uniform, the host PERMUTES x's sequence tiles per
core so the core's own query tiles always sit at EVEN 128-column positions
(p=0: natural order; p=1: pairwise swap). All causal structure is then
position-uniform; the one residual parity difference (whether the odd
neighbor tile is a past or future key) is data (maskB2 below).

Device algorithm per core (everything on the PE runs in bf16, 1 cycle/row
at any moving size; PSUM accumulates fp32):
  - x streams in as one batched DMA per 512-column chunk ([128, 8, 512]
    d-major tile). K^T/V^T/Q^T project with the d-contraction on
    partitions, accumulating over 8 d-chunks in PSUM; Q uses the same
    resident x tiles (own tiles = even 128-blocks, one strided-AP matmul)
    so x is read exactly once. Biases fold in during PSUM->SBUF eviction;
    the 1/sqrt(128) score scale is folded into Wq/bq on the host.
  - V^T transposes back to [key, head] via 4 PE transposes sharing one
    PSUM accumulation region, then one strided DVE copy into the Vm
    buffer whose 129th column is a constant 1 (memset once).
  - Scores S^T[k,q] = KT-tile.T @ QT-chunk. The causal mask is applied
    INSIDE the score accumulation group on the PE: for the two edge key
    tiles of each query block, a matmul with stationary -1e30*triangle
    (resp. the parity mask) and identity moving adds -1e30 above the
    diagonal. exp() then runs on the scalar engine straight out of PSUM
    with the key-padding bias per partition; its only dependency is the
    PE, so no extra semaphore ops are legalized in. Softmax
    max-subtraction is skipped: scores are ~N(0,1) by construction.
  - attn@V runs with the exp tile as the STATIONARY operand and
    [V | ones] (129 cols) as the moving operand: out[q,128:129] then IS
    the softmax denominator, accumulated for free, and the output comes
    out in [q, head] orientation. Two accumulation groups share each PSUM
    bank (single start/stop per bank; PSUM pending-zero semantics make
    the second group's first touch an overwrite).
  - Normalization is reciprocal + per-partition scalar multiply.
  - Projections for chunk j+1 are emission-interleaved into attention
    chunk j so the PE has filler while the scalar engine exps.
"""

import sys

sys.path.insert(0, "/opt/trn_rl_repo")

import numpy as np
import ml_dtypes

import concourse.bass as bass
import concourse.bacc as bacc
import concourse.tile as tile
from concourse import mybir
from concourse import bass_utils

F32 = mybir.dt.float32
BF16 = mybir.dt.bfloat16
B, S, D, H = 4, 4096, 1024, 128
NQ = S // 2          # queries owned per core (2048)
DC = D // 128        # 8 d-chunks
NSC = S // 512       # 8 seq chunks of 512
NKT = S // 128       # 32 key tile positions
NJ = 4               # attention chunks of 512 owned queries
VW = 132             # Vm row pitch (129 used)


def _build_program():
    nc = bacc.Bacc("TRN2", target_bir_lowering=False)

    xT_d = nc.dram_tensor("xT", [D, S], BF16, kind="ExternalInput")
    wqT_d = nc.dram_tensor("wqT", [128, DC * H], BF16, kind="ExternalInput")
    wkT_d = nc.dram_tensor("wkT", [128, DC * H], BF16, kind="ExternalInput")
    wvT_d = nc.dram_tensor("wvT", [128, DC * H], BF16, kind="ExternalInput")
    bq_d = nc.dram_tensor("bq", [H, 1], F32, kind="ExternalInput")
    bk_d = nc.dram_tensor("bk", [H, 1], F32, kind="ExternalInput")
    bv_d = nc.dram_tensor("bv", [H, 1], F32, kind="ExternalInput")
    ident_d = nc.dram_tensor("ident", [128, 128], BF16, kind="ExternalInput")
    triM_d = nc.dram_tensor("triM", [128, 128], BF16, kind="ExternalInput")
    mb2_d = nc.dram_tensor("maskB2", [128, 128], BF16, kind="ExternalInput")
    mb_d = nc.dram_tensor("mb", [128, NKT], F32, kind="ExternalInput")
    o_d = nc.dram_tensor("o", [NQ, H], F32, kind="ExternalOutput")

    with tile.TileContext(nc) as tc:
        with (
            tc.tile_pool(name="consts", bufs=1) as consts,
            tc.tile_pool(name="big", bufs=1) as big,
            tc.tile_pool(name="xp", bufs=7) as xp,
            tc.tile_pool(name="vstage", bufs=2) as vstage,
            tc.tile_pool(name="ptp", bufs=8) as ptp,
            tc.tile_pool(name="osb", bufs=4) as osb,
            tc.tile_pool(name="rp", bufs=4) as rp,
            tc.tile_pool(name="kvps", bufs=1, space="PSUM") as kvps,
            tc.tile_pool(name="scr", bufs=1, space="PSUM") as scr,
            tc.tile_pool(name="qps", bufs=1, space="PSUM") as qps,
            tc.tile_pool(name="sp", bufs=2, space="PSUM") as sp,
            tc.tile_pool(name="op", bufs=2, space="PSUM") as op,
        ):
            # ---- weights on the gpsimd queue; sync carries only x tiles so
            # the first x chunk starts transferring as early as possible ----
            w_sb = {}
            for name, dram in (("k", wkT_d), ("v", wvT_d), ("q", wqT_d)):
                t = consts.tile([128, DC, H], BF16, tag=f"w_{name}")
                nc.gpsimd.dma_start(out=t, in_=dram[:, :].rearrange("p (c h) -> p c h", c=DC))
                w_sb[name] = t
            # ---- small consts on the gpsimd queue (off critical path) ----
            ident = consts.tile([128, 128], BF16)
            nc.gpsimd.dma_start(out=ident, in_=ident_d[:, :])
            triM = consts.tile([128, 128], BF16)
            nc.gpsimd.dma_start(out=triM, in_=triM_d[:, :])
            maskB2 = consts.tile([128, 128], BF16)
            nc.gpsimd.dma_start(out=maskB2, in_=mb2_d[:, :])
            mb = consts.tile([128, NKT], F32)
            nc.gpsimd.dma_start(out=mb, in_=mb_d[:, :])
            b_sb = {}
            for name, dram in (("q", bq_d), ("k", bk_d), ("v", bv_d)):
                t = consts.tile([H, 1], F32, tag=f"b_{name}")
                nc.gpsimd.dma_start(out=t, in_=dram[:, :])
                b_sb[name] = t

            KT = big.tile([128, S], BF16, tag="KT")      # K^T [h, kpos]
            QT = big.tile([128, NQ], BF16, tag="QT")     # Q^T [h, own q]
            Vm = big.tile([128, NKT, VW], BF16, tag="Vm")  # [k, h | ones]

            # PE p-state warmup: the tensor engine only reaches full clock
            # after ~3us of continuous execution. Run garbage matmuls while
            # the first x chunk is still in flight so the real work starts
            # at speed. (Reads an unwritten tile; the result is discarded.)
            wsrc = big.tile([128, 512], BF16, tag="wsrc")
            nc.vector.memset(wsrc, 0.0)
            nc.vector.memset(Vm[:, :, 128:129], 1.0)
            for _ in range(18):
                wdst = sp.tile([128, 512], F32, tag="sp", name="wdst")
                nc.tensor.matmul(wdst, wsrc[:, 0:128], wsrc,
                                 start=True, stop=True)

            # ---- projection emission units ----
            # one unit = one (s-chunk, d-chunk) step or an eviction step;
            # attention chunks interleave these to keep the PE fed.
            cur = {}

            def proj_dma(sc, split=False):
                if split:
                    # dc-pair DMAs so the first matmul can start after ~1/4
                    # of the chunk has landed
                    tiles = []
                    for h in range(4):
                        xb = xp.tile([128, 2, 512], BF16, tag=f"xs{h}",
                                     name=f"xs{h}", bufs=1)
                        nc.sync.dma_start(
                            out=xb,
                            in_=xT_d[h * 256:(h + 1) * 256,
                                     sc * 512:(sc + 1) * 512].rearrange(
                                "(c p) s -> p c s", p=128),
                        )
                        tiles.append(xb)
                    cur[sc] = lambda dc: tiles[dc // 2][:, dc % 2, :]
                else:
                    xb = xp.tile([128, DC, 512], BF16, tag="xt")
                    eng = nc.sync
                    eng.dma_start(
                        out=xb,
                        in_=xT_d[:, sc * 512:(sc + 1) * 512].rearrange(
                            "(c p) s -> p c s", p=128),
                    )
                    cur[sc] = lambda dc, xb=xb: xb[:, dc, :]

            def proj_step(sc, dc):
                if dc == 0:
                    cur["kv"] = kvps.tile([128, 1024], F32, name="kvp")
                    cur["q"] = qps.tile([128, 256], F32, name="qp")
                kvp, qp = cur["kv"], cur["q"]
                xt = cur[sc](dc)
                st, sp_ = (dc == 0), (dc == DC - 1)
                nc.tensor.matmul(kvp[:, 0:512], w_sb["k"][:, dc, :], xt,
                                 start=st, stop=sp_)
                nc.tensor.matmul(kvp[:, 512:1024], w_sb["v"][:, dc, :], xt,
                                 start=st, stop=sp_)
                # own query tiles sit at even 128-positions: cols 0:128, 256:384
                xq = bass.AP(tensor=xt.tensor, offset=xt.offset,
                             ap=[list(xt.ap[0]), [256, 2], [1, 128]])
                nc.tensor.matmul(qp, w_sb["q"][:, dc, :], xq,
                                 start=st, stop=sp_)

            def proj_evict(sc):
                kvp, qp = cur["kv"], cur["q"]
                nc.vector.tensor_scalar_add(
                    KT[:, sc * 512:(sc + 1) * 512], kvp[:, 0:512], b_sb["k"])
                vst = vstage.tile([128, 512], BF16, tag="vst")
                nc.vector.tensor_scalar_add(vst, kvp[:, 512:1024], b_sb["v"])
                nc.vector.tensor_scalar_add(
                    QT[:, sc * 256:(sc + 1) * 256], qp, b_sb["q"])
                cur["vst"] = vst

            def proj_vt(sc, i):
                # 4 transposes share one PSUM region (single start/stop group)
                if i == 0:
                    cur["tq"] = scr.tile([128, 512], BF16, name="tq")
                nc.tensor.matmul(
                    cur["tq"][:, i * 128:(i + 1) * 128],
                    cur["vst"][:, i * 128:(i + 1) * 128], ident,
                    is_transpose=True, start=(i == 0), stop=(i == 3))
                if i == 3:
                    nc.vector.tensor_copy(
                        Vm[:, 4 * sc:4 * sc + 4, 0:128],
                        cur["tq"].rearrange("p (a b) -> p a b", a=4))

            def proj_units(sc):
                for dc in range(DC):
                    yield lambda dc=dc: proj_step(sc, dc)
                yield lambda: proj_evict(sc)
                for i in range(4):
                    yield lambda i=i: proj_vt(sc, i)

            # ---- attention chunk j over owned query cols [512j, 512j+512) ----
            # q-block qi (0..3) is own tile t=4j+qi at key-position 8j+2qi;
            # PV for (qi, kt) needed for kt <= 8j+2qi+1. The causal edge is
            # applied pre-exp on the PE: kt==8j+2qi adds -1e30 above the
            # diagonal (triM), kt==8j+2qi+1 adds the parity mask (maskB2:
            # -1e30 everywhere for p=0 where the neighbor is a future key,
            # 0 for p=1 where it is a past key).
            def attention(j, filler):
                n_kt = 8 * j + 8
                opsA = op.tile([128, 258], F32, tag="o2")   # qi 0,1
                opsB = op.tile([128, 258], F32, tag="o2")   # qi 2,3
                pts = [None] * n_kt

                def score_exp(kt):
                    hi_only = kt > 8 * j + 3
                    w = 256 if hi_only else 512
                    qoff = j * 512 + (256 if hi_only else 0)
                    edge = kt >= 8 * j
                    spsum = sp.tile([128, 512], F32, tag="sp")
                    nc.tensor.matmul(
                        spsum[:, 0:w], KT[:, kt * 128:(kt + 1) * 128],
                        QT[:, qoff:qoff + w], start=True, stop=not edge)
                    if edge:
                        qi_e, c = (kt - 8 * j) // 2, (kt - 8 * j) % 2
                        lo = qi_e * 128 - (256 if hi_only else 0)
                        nc.tensor.matmul(
                            spsum[:, lo:lo + 128],
                            triM if c == 0 else maskB2, ident,
                            start=False, stop=True)
                    pt = ptp.tile([128, 512], BF16, tag="pt")
                    nc.scalar.activation(
                        pt[:, 0:w], spsum[:, 0:w],
                        mybir.ActivationFunctionType.Exp,
                        bias=mb[:, kt:kt + 1], scale=1.0)
                    pts[kt] = pt

                def pv(kt):
                    hi_only = kt > 8 * j + 3
                    qi_min = max(0, -(-(kt - 8 * j - 1) // 2))
                    for qi in range(qi_min, 4):
                        lo = qi * 128 - (256 if hi_only else 0)
                        ops = opsA if qi < 2 else opsB
                        col = (qi % 2) * 129
                        nc.tensor.matmul(
                            ops[:, col:col + 129], pts[kt][:, lo:lo + 128],
                            Vm[:, kt, 0:129],
                            start=(kt == 0 and qi % 2 == 0),
                            stop=(qi == 1 and kt == 8 * j + 3)
                            or (qi == 3 and kt == n_kt - 1),
                        )

                def epilogue(qi):
                    ops = opsA if qi < 2 else opsB
                    col = (qi % 2) * 129
                    r = rp.tile([128, 1], F32, tag="r")
                    nc.vector.reciprocal(r, ops[:, col + 128:col + 129])
                    o_sb = osb.tile([128, 128], F32, tag="o")
                    nc.vector.tensor_scalar_mul(o_sb, ops[:, col:col + 128], r)
                    row = (4 * j + qi) * 128
                    nc.sync.dma_start(out=o_d[row:row + 128, :], in_=o_sb)

                # lag-2 software pipeline: pv(kt) runs two score steps after
                # score_exp(kt), so the PE never waits on the scalar engine's
                # exp latency in steady state. Filler absorbs the one
                # score(2)-waits-exp(0) PSUM-reuse bubble at chunk start.
                score_exp(0)
                score_exp(1)
                for f in filler.pop_units(2):
                    f()
                for kt in range(2, n_kt):
                    score_exp(kt)
                    for f in filler.pop_units(4 if j == 0 else 2):
                        f()
                    pv(kt - 2)
                    if kt - 2 == 8 * j + 3:
                        epilogue(0)
                        epilogue(1)
                    elif kt - 2 == 8 * j + 5:
                        epilogue(2)
                pv(n_kt - 2)
                pv(n_kt - 1)
                epilogue(3)

            class Filler:
                def __init__(self):
                    self.units = []

                def add(self, sc):
                    self.units.extend(proj_units(sc))

                def pop_units(self, k):
                    for _ in range(k):
                        if self.units:
                            yield self.units.pop(0)

                def drain(self):
                    yield from self.pop_units(len(self.units))

            filler = Filler()
            proj_dma(0, split=True)
            for sc in range(1, NSC):
                proj_dma(sc)
            filler.add(0)
            filler.add(1)
            for f in filler.drain():
                f()
            for j in range(NJ):
                if 2 * j + 2 < NSC:
                    filler.add(2 * j + 2)
                    filler.add(2 * j + 3)
                attention(j, filler)
                for f in filler.drain():
                    f()
    nc.compile()
    return nc


_NC_CACHE = {}


def _get_program():
    if "nc" not in _NC_CACHE:
        _NC_CACHE["nc"] = _build_program()
    return _NC_CACHE["nc"]


def _make_in_maps(x, mask, Wq, bq, Wk, bk, Wv, bv):
    x = np.asarray(x, np.float32)
    mask = np.asarray(mask)
    scale = 1.0 / np.sqrt(np.float32(H))
    bf16 = ml_dtypes.bfloat16
    NEG = np.float32(-1.0e30)

    def pack_w(w):
        # [H,D] -> w.T [D,H] -> partition-major [128, DC*H] for a single
        # contiguous-burst DMA into the SBUF weight tile
        wT = np.asarray(w, np.float32).T.reshape(DC, 128, H)
        return np.ascontiguousarray(
            wT.transpose(1, 0, 2).reshape(128, DC * H).astype(bf16))

    wqT = pack_w(np.asarray(Wq, np.float32) * scale)
    wkT = pack_w(Wk)
    wvT = pack_w(Wv)
    bq_c = (np.asarray(bq, np.float32) * scale).reshape(H, 1).copy()
    bk_c = np.asarray(bk, np.float32).reshape(H, 1).copy()
    bv_c = np.asarray(bv, np.float32).reshape(H, 1).copy()
    ident = np.eye(128, dtype=bf16)
    # score += triM.T[k, q']: -1e30 where q' < k (strict upper as [q', k])
    triM = (NEG * np.triu(np.ones((128, 128), np.float32), 1)).astype(bf16)

    in_maps = []
    for c in range(8):
        b, p = c // 2, c % 2
        # permuted tile order: even positions = own tiles (parity p)
        perm = np.arange(NKT).reshape(-1, 2)
        if p == 1:
            perm = perm[:, ::-1]
        perm = perm.reshape(-1)                                # pos -> global tile
        xT = x[b].T.reshape(D, NKT, 128)[:, perm, :].reshape(D, S)
        maskB2 = (np.full((128, 128), NEG) if p == 0
                  else np.zeros((128, 128), np.float32))
        mb = np.where(mask[b] != 0, np.float32(0.0), NEG)
        mb = np.ascontiguousarray(mb.reshape(NKT, 128)[perm, :].T)
        in_maps.append({
            "xT": np.ascontiguousarray(xT.astype(bf16)),
            "wqT": wqT, "wkT": wkT, "wvT": wvT,
            "bq": bq_c, "bk": bk_c, "bv": bv_c,
            "ident": ident, "triM": triM, "maskB2": maskB2.astype(bf16),
            "mb": mb,
        })
    return in_maps


def _install_ntff_hook():
    # the trimmed antenv package lacks axon_hooks; recreate it and wire the
    # ctypes NTFF profiling hook from trn_agent_boot so trace=True works
    import types
    if "antenv.axon_hooks" in sys.modules:
        return
    import antenv
    mod = types.ModuleType("antenv.axon_hooks")
    _hook = [None]
    mod.set_axon_ntff_profile_hook = lambda h: _hook.__setitem__(0, h)
    mod.get_axon_ntff_profile_hook = lambda: _hook[0]
    sys.modules["antenv.axon_hooks"] = mod
    antenv.axon_hooks = mod
    from trn_agent_boot.trn_boot import _ntff_profile_via_ctypes
    mod.set_axon_ntff_profile_hook(
        _ntff_profile_via_ctypes("/opt/axon/libaxon_pjrt.so"))


def run(inputs, trace=False, tmpdir=None):
    if trace:
        try:
            _install_ntff_hook()
        except Exception as e:
            print("ntff hook install failed:", e)
    nc = _get_program()
    in_maps = _make_in_maps(**inputs)
    res = bass_utils.run_bass_kernel_spmd(
        nc, in_maps, core_ids=list(range(8)), trace=trace, tmpdir=tmpdir)
    out = np.empty((B, S, H), np.float32)
    for c in range(8):
        b, p = c // 2, c % 2
        o = res.results[c]["o"]                                # [NQ, H]
        for t in range(16):
            g = 2 * t + p
            out[b, g * 128:(g + 1) * 128, :] = o[t * 128:(t + 1) * 128, :]
    return out, res


def kernel(**inputs) -> np.ndarray:
    out, _ = run(inputs, trace=False)
    return out
